# revision 29
# baseline (speedup 1.0000x reference)
"""Trainium2 Bass kernel for AttentiveNonLocalBlock2D (v2, fp8 DoubleRow).

Per-core SPMD over 8 NeuronCores, sequence-parallel over N=H*W:
  Phase A: 3x stride-2 conv gating unit (fp16 PE, lrelu as one max-STT op)
    -> bilinear x8 upsample (f16, DVE/Pool row split) -> sigmoid gate (ACT)
    -> x_gated f16; projections phi/theta (fp16 PE) quantized to fp8e4;
    G^T = x_gated^T (W_w g_w)^T in fp16 -> fp8e4.
  Pass 1: score tiles f^T[m_tile, n_chunk] via fp8 DoubleRow matmuls
    (zero-companion theta trick: lhsT broadcast, rhs [theta|zeros]);
    exp(f-5) via ACT directly into an fp8e4 SBUF cache; softmax partials
    Z[m] via ACT accum_out / DVE tensor_reduce (AllReduce over cores).
  Pass 2: fp8 DoubleRow over adjacent m-tile pairs, all 72 m-tiles
    accumulated into one PSUM group per 256-col chunk; G is pre-scaled by
    2^GK/Z, final out = psum * 2^-GK + x_gated, DMA per chunk.
"""

import sys

if "/opt/trn_rl_repo" not in sys.path:
    sys.path.insert(0, "/opt/trn_rl_repo")

import numpy as np

NCORES = 8
C, CI, H, W = 64, 32, 96, 96
N = H * W            # 9216
CH = N // NCORES     # 1152 pixels per core
MT = N // 128        # 72 m-tiles
HALF = MT // 2       # 36 (also the s_cacheA tile count)
NA = HALF
SUBS = ((0, 512), (512, 512), (1024, 128))        # 512-col chunks (proj)
SUBS256 = ((0, 256), (256, 256), (512, 256), (768, 256), (1024, 128))
DELTA = 5.5133       # theta row-33 constant: fps = f + DELTA (Schraudolph offset)
EXP_BIAS = -5.0 - DELTA   # ACT tiles: exp(fps + EXP_BIAS) = exp(f - 5)
SCH_SCALE = 5.770780      # 4*log2(e): e5m2 bits = fps * SCH_SCALE (floor, clamp 0)
GK = 256.0           # G pre-scale folded into 1/Z; undone in final STT

_compiled = {}


def _build(single=False):
    import concourse.bacc as bacc
    import concourse.bass as bass
    import concourse.mybir as mybir
    import concourse.tile as tile

    f16 = mybir.dt.float16
    f32 = mybir.dt.float32
    f8 = mybir.dt.float8e4
    f8e5 = mybir.dt.float8e5
    i8 = mybir.dt.int8
    AF = mybir.ActivationFunctionType
    ALU = mybir.AluOpType
    PM = mybir.MatmulPerfMode

    nc = bacc.Bacc("TRN2", target_bir_lowering=False, debug=False,
                   num_devices=1 if single else NCORES)

    xpad_io = nc.dram_tensor("xpad", [C, 98, 98], f16, kind="ExternalInput")
    x16_io = nc.dram_tensor("x16", [C, N], f16, kind="ExternalInput")
    w1_io = nc.dram_tensor("w1", [C, 9 * C], f16, kind="ExternalInput")
    w2_io = nc.dram_tensor("w2", [C, 9 * C], f16, kind="ExternalInput")
    w3_io = nc.dram_tensor("w3", [C, 9 * C], f16, kind="ExternalInput")
    twT_io = nc.dram_tensor("twT", [C, CI], f16, kind="ExternalInput")
    pwT_io = nc.dram_tensor("pwT", [C, CI], f16, kind="ExternalInput")
    gw_io = nc.dram_tensor("gw", [CI, C], f16, kind="ExternalInput")
    WwT_io = nc.dram_tensor("WwT", [CI, C], f16, kind="ExternalInput")
    xch_io = nc.dram_tensor("xch", [C, CH], f16, kind="ExternalInput")
    ones_io = nc.dram_tensor("ones_row", [1, N], f16, kind="ExternalInput")
    delta_io = nc.dram_tensor("delta_row", [1, CH], f16, kind="ExternalInput")
    out_io = nc.dram_tensor("out", [C, CH], f32, kind="ExternalOutput")

    with tile.TileContext(nc) as tc:
        with tc.tile_pool(name="persist", bufs=1) as pp, \
             tc.tile_pool(name="dram", bufs=1, space="DRAM") as dp:
            zsum = pp.tile([128, MT], f32)
            nb5 = pp.tile([128, 1], f32)
            nc.gpsimd.memset(nb5[:], EXP_BIAS)
            shared = {} if single else {"addr_space": "Shared"}
            zinA = dp.tile([128, HALF], f32)
            zoutA = dp.tile([NCORES, 128, HALF], f32, **shared)
            zinB = dp.tile([128, HALF], f32)
            zoutB = dp.tile([NCORES, 128, HALF], f32, **shared)

            with tc.tile_pool(name="hand", bufs=1) as hp:
                phi16 = hp.tile([CI + 1, N], f16)
                th16 = hp.tile([CI + 1, CH], f16)
                G16 = hp.tile([128, MT * C], f16)
                Gw3 = G16[:].rearrange("p (j c) -> p j c", c=C)
                G8 = hp.tile([128, MT * C], f8)
                G3 = G8[:].rearrange("p (j c) -> p j c", c=C)
                xg16 = hp.tile([C, N], f16)
                xres = hp.tile([C, CH], f16)
                outsb = hp.tile([C, CH], f32)
                s_cache = hp.tile([128, MT * CH], f8e5)
                s3 = s_cache[:].rearrange("p (j n) -> p j n", n=CH)

                def s_sl(j):
                    return s_cache[:, j * CH:(j + 1) * CH]

                def s_pair(jl, o0, w):
                    # [128, 2, w] rhs for a DoubleRow pass-2 pair
                    return s3[:, jl:jl + 2, o0:o0 + w]

                p1ps = None

                def allreduce(zi, zo, jsl):
                    nc.sync.dma_start(zi[:], zsum[:, jsl])
                    if single:
                        for sh in range(NCORES):
                            nc.sync.dma_start(zo[sh], zi[:])
                    else:
                        nc.gpsimd.collective_compute(
                            "AllGather", ALU.bypass,
                            replica_groups=[list(range(NCORES))],
                            ins=[zi.opt()], outs=[zo.opt()])

                def scale_G(hh, zo, eng):
                    # gather shards -> sum -> reciprocal -> scale G half
                    zr = hp.tile([128, NCORES, HALF], f32, tag="zr", name="zr")
                    nc.sync.dma_start(zr[:], zo[:].rearrange("s p h -> p s h"))
                    zf = hp.tile([128, HALF], f32, tag="zf", name="zf",
                                 bufs=2)
                    nc.vector.tensor_reduce(
                        zf[:], zr[:].rearrange("p s h -> p h s"),
                        axis=mybir.AxisListType.X, op=ALU.add)
                    rz = hp.tile([128, HALF], f32, tag="rz", name="rz",
                                 bufs=2)
                    nc.vector.reciprocal(rz[:], zf[:])
                    rz2 = hp.tile([128, HALF], f32, tag="rz2", name="rz2",
                                  bufs=2)
                    nc.vector.tensor_scalar(rz2[:], rz[:], GK, None,
                                            op0=ALU.mult)
                    rzb = rz2[:].unsqueeze(-1).to_broadcast((128, HALF, C))
                    j0 = hh * HALF
                    eng.tensor_mul(
                        G3[:, j0:j0 + HALF, :], Gw3[:, j0:j0 + HALF, :], rzb)

                def pass1_tile(j):
                    fps = p1ps.tile([128, CH], f32, tag="fps", name="fps")
                    for o0, w in SUBS:
                        nc.tensor.matmul(fps[:, o0:o0 + w],
                                         phi16[:, j * 128:(j + 1) * 128],
                                         th16[:, o0:o0 + w],
                                         start=True, stop=True)
                    ssl = s_sl(j)
                    if j % 3 == 2:
                        # Schraudolph on DVE: e5m2 bits = floor(max(fps*S, 0))
                        nc.vector.tensor_scalar(ssl.bitcast(i8), fps[:],
                                                SCH_SCALE, 0.0,
                                                op0=ALU.mult, op1=ALU.max)
                        nc.vector.tensor_reduce(
                            zsum[:, j:j + 1], ssl,
                            axis=mybir.AxisListType.X, op=ALU.add)
                    else:
                        nc.scalar.activation(ssl, fps[:], AF.Exp,
                                             bias=nb5[:], scale=1.0,
                                             accum_out=zsum[:, j:j + 1])

                # ========== PHASE A + PASS 1 (share p1ps PSUM) ==========
                p1ps = tc.alloc_tile_pool(name="p1ps", bufs=2, space="PSUM")
                if True:
                    pm = tc.alloc_tile_pool(name="mid", bufs=1)
                    yh = pm.tile([C, N], f16)
                    yh3 = yh[:].rearrange("c (h w) -> c h w", h=H)

                    # --- A1: convs + upsample ---
                    with tc.tile_pool(name="pa1", bufs=1) as pa, \
                         tc.tile_pool(name="paps1", bufs=2, space="PSUM") as paps:
                        w1sb = pa.tile([C, 9 * C], f16)
                        nc.sync.dma_start(w1sb[:], w1_io[:])
                        w2sb = pa.tile([C, 9 * C], f16)
                        nc.sync.dma_start(w2sb[:], w2_io[:])
                        w3sb = pa.tile([C, 9 * C], f16)
                        nc.sync.dma_start(w3sb[:], w3_io[:])
                        xpad = pa.tile([C, 98, 98], f16)
                        for b in range(4):
                            r0, r1 = 26 * b, min(26 * b + 26, 98)
                            nc.sync.dma_start(xpad[:, r0:r1, :],
                                              xpad_io[:, r0:r1, :])
                        twTsb = hp.tile([C, CI], f16)
                        nc.sync.dma_start(twTsb[:], twT_io[:])
                        pwTsb = hp.tile([C, CI], f16)
                        nc.sync.dma_start(pwTsb[:], pwT_io[:])
                        gwsb = hp.tile([CI, C], f16)
                        nc.sync.dma_start(gwsb[:], gw_io[:])
                        WwTsb = hp.tile([CI, C], f16)
                        nc.sync.dma_start(WwTsb[:], WwT_io[:])
                        xchsb = hp.tile([C, CH], f16)
                        nc.sync.dma_start(xchsb[:], xch_io[:])

                        # conv1: 96x96 -> 48x48, stride 2, pad 1, lrelu(0.2)
                        y1p = pa.tile([C, 50, 50], f16)
                        nc.gpsimd.memset(y1p[:, 0:1, :], 0.0)
                        nc.gpsimd.memset(y1p[:, 49:50, :], 0.0)
                        nc.gpsimd.memset(y1p[:, :, 0:1], 0.0)
                        nc.gpsimd.memset(y1p[:, :, 49:50], 0.0)
                        for g in range(6):
                            ps1 = paps.tile([C, 8, 48], f32, tag="cv", name="ps1")
                            for t in range(9):
                                dy, dx = t // 3, t % 3
                                nc.tensor.matmul(
                                    ps1[:], w1sb[:, t * C:(t + 1) * C],
                                    xpad[:, 16 * g + dy: 16 * g + dy + 16: 2,
                                         dx: dx + 96: 2],
                                    start=(t == 0), stop=(t == 8))
                            ab1 = pa.tile([C, 8 * 48], f32, tag="ab1",
                                          name="ab1")
                            nc.scalar.activation(ab1[:], ps1[:], AF.Abs,
                                                 scale=0.4)
                            nc.vector.scalar_tensor_tensor(
                                y1p[:, 1 + 8 * g: 9 + 8 * g, 1:49], ps1[:], 0.6,
                                ab1[:], op0=ALU.mult, op1=ALU.add)

                        # conv2: 48x48 -> 24x24
                        y2p = pa.tile([C, 26, 26], f16)
                        nc.gpsimd.memset(y2p[:, 0:1, :], 0.0)
                        nc.gpsimd.memset(y2p[:, 25:26, :], 0.0)
                        nc.gpsimd.memset(y2p[:, :, 0:1], 0.0)
                        nc.gpsimd.memset(y2p[:, :, 25:26], 0.0)
                        for g in range(2):
                            ps2 = paps.tile([C, 12, 24], f32, tag="cv", name="ps2")
                            for t in range(9):
                                dy, dx = t // 3, t % 3
                                nc.tensor.matmul(
                                    ps2[:], w2sb[:, t * C:(t + 1) * C],
                                    y1p[:, 24 * g + dy: 24 * g + dy + 24: 2,
                                        dx: dx + 48: 2],
                                    start=(t == 0), stop=(t == 8))
                            ab2 = pa.tile([C, 12 * 24], f32, tag="ab2",
                                          name="ab2")
                            nc.scalar.activation(ab2[:], ps2[:], AF.Abs,
                                                 scale=0.4)
                            nc.vector.scalar_tensor_tensor(
                                y2p[:, 1 + 12 * g: 13 + 12 * g, 1:25], ps2[:], 0.6,
                                ab2[:], op0=ALU.mult, op1=ALU.add)

                        # conv3: 24x24 -> 12x12 (no activation)
                        ps3 = paps.tile([C, 12, 12], f32, tag="cv", name="ps3")
                        for t in range(9):
                            dy, dx = t // 3, t % 3
                            nc.tensor.matmul(
                                ps3[:], w3sb[:, t * C:(t + 1) * C],
                                y2p[:, dy: dy + 24: 2, dx: dx + 24: 2],
                                start=(t == 0), stop=(t == 8))
                        y3v = pa.tile([C, 14, 12], f16)
                        nc.vector.tensor_copy(y3v[:, 1:13, :], ps3[:])
                        nc.vector.tensor_copy(y3v[:, 0:1, :], ps3[:, 0:1, :])
                        nc.vector.tensor_copy(y3v[:, 13:14, :], ps3[:, 11:12, :])

                        # bilinear x8 vertical: out[8k+r] = X + b_r * (Y - X)
                        yvp = pa.tile([C, 96, 14], f16)
                        dv = pa.tile([C, 13, 12], f16)
                        nc.vector.tensor_sub(dv[:], y3v[:, 1:14, :], y3v[:, 0:13, :])
                        for r in range(8):
                            t = (r + 0.5) / 8 - 0.5
                            kr, b = (0, 1 + t) if r < 4 else (1, t)
                            nc.vector.scalar_tensor_tensor(
                                yvp[:, r:96:8, 1:13], dv[:, kr:kr + 12, :], float(b),
                                y3v[:, kr:kr + 12, :], op0=ALU.mult, op1=ALU.add)
                        nc.vector.tensor_copy(yvp[:, :, 0:1], yvp[:, :, 1:2])
                        nc.vector.tensor_copy(yvp[:, :, 13:14], yvp[:, :, 12:13])

                        # bilinear x8 horizontal -> yh [C, 9216] f16.
                        # Row-split: DVE rows 0-47, Pool rows 48-95 so early
                        # sigmoid chunks unblock as soon as DVE's half lands.
                        dh = pa.tile([C, 96, 13], f16)
                        nc.vector.tensor_sub(dh[:], yvp[:, :, 1:14], yvp[:, :, 0:13])
                        for hf in (0, 1):
                            h0 = 48 * hf
                            for r in range(8):
                                t = (r + 0.5) / 8 - 0.5
                                kr, b = (0, 1 + t) if r < 4 else (1, t)
                                nc.vector.scalar_tensor_tensor(
                                    yh3[:, h0:h0 + 48, r:96:8],
                                    dh[:, h0:h0 + 48, kr:kr + 12], float(b),
                                    yvp[:, h0:h0 + 48, kr:kr + 12],
                                    op0=ALU.mult, op1=ALU.add)

                    # --- A2: gate, x_gated, projections ---
                    with tc.tile_pool(name="pa2", bufs=1) as pa, \
                         tc.tile_pool(name="paps2", bufs=1, space="PSUM") as paps:

                        nc.sync.dma_start(th16[CI:CI + 1, :], delta_io[:])
                        nc.sync.dma_start(phi16[CI:CI + 1, :], ones_io[:])

                        # early own-chunk path: sigmoid -> x_gated (residual +
                        # theta source); unblocks pass 1 early
                        yhc = pa.tile([C, CH], f16)
                        with tc.tile_critical():
                            pid = nc.vector.partition_id()
                            col0 = pid * CH
                            nc.vector.tensor_copy(yhc[:], yh[:, bass.ds(col0, CH)])
                        gtc = pa.tile([C, CH], f16)
                        nc.scalar.activation(gtc[:], yhc[:], AF.Sigmoid)
                        nc.gpsimd.tensor_mul(xres[:], gtc[:], xchsb[:])

                        # theta chunk [CI, CH] -> fp8 (with zero companion)
                        for o0, w in SUBS:
                            tps = paps.tile([CI, 512], f32, tag="prj", name="tps",
                                            bufs=2)
                            nc.tensor.matmul(tps[:, 0:w], twTsb[:],
                                             xres[:, o0:o0 + w],
                                             start=True, stop=True)
                            nc.vector.tensor_copy(th16[0:CI, o0:o0 + w],
                                                  tps[:, 0:w])

                        # E^T = gw^T WwT [C, C]
                        eps = paps.tile([C, 512], f32, tag="prj", name="eps",
                                        bufs=2)
                        nc.tensor.matmul(eps[:, 0:C], gwsb[:], WwTsb[:],
                                         start=True, stop=True)
                        ET16 = pa.tile([C, C], f16)
                        nc.vector.tensor_copy(ET16[:], eps[:, 0:C])

                        def gt_group(gg):
                            gps = paps.tile([128, 8 * C], f32, tag="prj",
                                            name="gps", bufs=2)
                            for u in range(8):
                                j = gg * 8 + u
                                nc.tensor.matmul(gps[:, u * C:(u + 1) * C],
                                                 xg16[:, j * 128:(j + 1) * 128],
                                                 ET16[:], start=True, stop=True)
                            nc.vector.tensor_copy(
                                G16[:, gg * 8 * C:(gg + 1) * 8 * C], gps[:])

                        # pipeline over 18 512-col chunks:
                        #   sigmoid -> x_gated -> phi -> fp8; interleave
                        #   G-groups and the first NA pass-1 tiles
                        for i in range(18):
                            sl = slice(i * 512, (i + 1) * 512)
                            xc = hp.tile([C, 512], f16, tag="xc", name="xc",
                                         bufs=4)
                            nc.sync.dma_start(xc[:], x16_io[:, sl])
                            gt = pa.tile([C, 512], f16, tag="gt", name="gt",
                                         bufs=3)
                            nc.scalar.activation(gt[:], yh[:, sl], AF.Sigmoid)
                            nc.vector.tensor_mul(xg16[:, sl], gt[:], xc[:])

                            pps = paps.tile([CI, 512], f32, tag="prj",
                                            name="pps", bufs=2)
                            nc.tensor.matmul(pps[:], pwTsb[:], xg16[:, sl],
                                             start=True, stop=True)
                            nc.vector.tensor_copy(phi16[0:CI, sl], pps[:])

                            if i % 2 == 1:
                                gt_group((i - 1) // 2)
                        for j in range(NA):
                            pass1_tile(j)

                    # ---- PASS 1 main + early pass-2 cols 0-1023 ----
                    pm.release()
                    p2e = tc.alloc_tile_pool(name="p2eps", bufs=1,
                                             space="PSUM")
                    yc01 = [None, None]

                    def pass2_pair01(u, hh, first, last):
                        jl = 2 * u + hh * HALF
                        for ci in (0, 1):
                            nc.tensor.matmul(
                                yc01[ci][:], G3[:, jl:jl + 2, :],
                                s_pair(jl, 512 * ci, 512),
                                start=first, stop=last,
                                perf_mode=PM.DoubleRow, skip_group_check=True)

                    for j in range(NA, MT):
                        pass1_tile(j)
                        if j == NA:
                            allreduce(zinA, zoutA, slice(0, HALF))
                            scale_G(0, zoutA, nc.gpsimd)
                            yc01[0] = p2e.tile([64, 512], f32, name="yc0")
                            yc01[1] = p2e.tile([64, 512], f32, name="yc1")
                        if j >= NA + 2 and j % 2 == 0:
                            u = (j - NA - 2) // 2
                            pass2_pair01(u, 0, first=(u == 0), last=False)
                    pass2_pair01(17, 0, first=False, last=False)
                    allreduce(zinB, zoutB, slice(HALF, MT))
                    scale_G(1, zoutB, nc.vector)
                    for u in range(HALF // 2):
                        pass2_pair01(u, 1, first=False,
                                     last=(u == HALF // 2 - 1))
                    for ci in (0, 1):
                        o0 = 512 * ci
                        nc.vector.scalar_tensor_tensor(
                            outsb[:, o0:o0 + 512], yc01[ci][:], 1.0 / GK,
                            xres[:, o0:o0 + 512], op0=ALU.mult, op1=ALU.add)
                        nc.sync.dma_start(out_io[:, o0:o0 + 512],
                                          outsb[:, o0:o0 + 512])
                    p2e.release()
                    p1ps.release()

                    # cols 1024-1151 tail
                    p2l = tc.alloc_tile_pool(name="p2lps", bufs=1,
                                             space="PSUM")
                    yc2 = p2l.tile([64, 128], f32, name="yc2")
                    for hh in (0, 1):
                        for u in range(HALF // 2):
                            jl = 2 * u + hh * HALF
                            nc.tensor.matmul(
                                yc2[:], G3[:, jl:jl + 2, :],
                                s_pair(jl, 1024, 128),
                                start=(hh == 0 and u == 0),
                                stop=(hh == 1 and u == HALF // 2 - 1),
                                perf_mode=PM.DoubleRow, skip_group_check=True)
                    nc.vector.scalar_tensor_tensor(
                        outsb[:, 1024:1152], yc2[:], 1.0 / GK,
                        xres[:, 1024:1152], op0=ALU.mult, op1=ALU.add)
                    nc.sync.dma_start(out_io[:, 1024:1152],
                                      outsb[:, 1024:1152])
                    p2l.release()

    nc.compile()
    return nc


def get_program():
    if "nc" not in _compiled:
        _compiled["nc"] = _build()
    return _compiled["nc"]


def make_in_maps(inputs):
    f16 = np.float16
    x = np.asarray(inputs["x"], np.float32).reshape(C, H, W)
    xflat = np.ascontiguousarray(x.reshape(C, N))
    xpad = np.zeros((C, 98, 98), f16)
    xpad[:, 1:97, 1:97] = x.astype(f16)

    def conv_w(w):
        # [o, i, dy, dx] -> [i, (dy dx), o]
        return np.ascontiguousarray(
            np.asarray(w, np.float32).transpose(1, 2, 3, 0).reshape(C, 9 * C)
        ).astype(f16)

    base = {
        "xpad": xpad,
        "ones_row": np.ones((1, N), f16),
        "delta_row": np.full((1, CH), DELTA, f16),
        "x16": xflat.astype(f16),
        "w1": conv_w(inputs["d1_w"]),
        "w2": conv_w(inputs["d2_w"]),
        "w3": conv_w(inputs["d3_w"]),
        "twT": np.ascontiguousarray(
            np.asarray(inputs["th_w"], np.float32)[:, :, 0, 0].T).astype(f16),
        "pwT": np.ascontiguousarray(
            np.asarray(inputs["ph_w"], np.float32)[:, :, 0, 0].T).astype(f16),
        "gw": np.ascontiguousarray(
            np.asarray(inputs["g_w"], np.float32)[:, :, 0, 0]).astype(f16),
        "WwT": np.ascontiguousarray(
            np.asarray(inputs["W_w"], np.float32)[:, :, 0, 0].T).astype(f16),
    }
    in_maps = []
    for k in range(NCORES):
        m = dict(base)
        m["xch"] = np.ascontiguousarray(
            xflat[:, k * CH:(k + 1) * CH]).astype(f16)
        in_maps.append(m)
    return in_maps


def kernel(**inputs):
    from concourse import bass_utils

    nc = get_program()
    in_maps = make_in_maps(inputs)
    res = bass_utils.run_bass_kernel_spmd(nc, in_maps,
                                          core_ids=list(range(NCORES)))
    out = np.concatenate([res.results[k]["out"] for k in range(NCORES)], axis=1)
    return out.reshape(1, C, H, W).astype(np.float32)


# revision 30
# speedup vs baseline: 1.0036x; 1.0036x over previous
"""Trainium2 Bass kernel for AttentiveNonLocalBlock2D (v2, fp8 DoubleRow).

Per-core SPMD over 8 NeuronCores, sequence-parallel over N=H*W:
  Phase A: 3x stride-2 conv gating unit (fp16 PE, lrelu as one max-STT op)
    -> bilinear x8 upsample (f16, DVE/Pool row split) -> sigmoid gate (ACT)
    -> x_gated f16; projections phi/theta (fp16 PE) quantized to fp8e4;
    G^T = x_gated^T (W_w g_w)^T in fp16 -> fp8e4.
  Pass 1: score tiles f^T[m_tile, n_chunk] via fp8 DoubleRow matmuls
    (zero-companion theta trick: lhsT broadcast, rhs [theta|zeros]);
    exp(f-5) via ACT directly into an fp8e4 SBUF cache; softmax partials
    Z[m] via ACT accum_out / DVE tensor_reduce (AllReduce over cores).
  Pass 2: fp8 DoubleRow over adjacent m-tile pairs, all 72 m-tiles
    accumulated into one PSUM group per 256-col chunk; G is pre-scaled by
    2^GK/Z, final out = psum * 2^-GK + x_gated, DMA per chunk.
"""

import sys

if "/opt/trn_rl_repo" not in sys.path:
    sys.path.insert(0, "/opt/trn_rl_repo")

import numpy as np

NCORES = 8
C, CI, H, W = 64, 32, 96, 96
N = H * W            # 9216
CH = N // NCORES     # 1152 pixels per core
MT = N // 128        # 72 m-tiles
HALF = MT // 2       # 36 (also the s_cacheA tile count)
NA = HALF
SUBS = ((0, 512), (512, 512), (1024, 128))        # 512-col chunks (proj)
SUBS256 = ((0, 256), (256, 256), (512, 256), (768, 256), (1024, 128))
DELTA = 5.5133       # theta row-33 constant: fps = f + DELTA (Schraudolph offset)
EXP_BIAS = -5.0 - DELTA   # ACT tiles: exp(fps + EXP_BIAS) = exp(f - 5)
SCH_SCALE = 5.770780      # 4*log2(e): e5m2 bits = fps * SCH_SCALE (floor, clamp 0)
GK = 256.0           # G pre-scale folded into 1/Z; undone in final STT

_compiled = {}


def _build(single=False):
    import concourse.bacc as bacc
    import concourse.bass as bass
    import concourse.mybir as mybir
    import concourse.tile as tile

    f16 = mybir.dt.float16
    f32 = mybir.dt.float32
    f8 = mybir.dt.float8e4
    f8e5 = mybir.dt.float8e5
    i8 = mybir.dt.int8
    AF = mybir.ActivationFunctionType
    ALU = mybir.AluOpType
    PM = mybir.MatmulPerfMode

    nc = bacc.Bacc("TRN2", target_bir_lowering=False, debug=False,
                   num_devices=1 if single else NCORES)

    xpad_io = nc.dram_tensor("xpad", [C, 98, 98], f16, kind="ExternalInput")
    x16_io = nc.dram_tensor("x16", [C, N], f16, kind="ExternalInput")
    w1_io = nc.dram_tensor("w1", [C, 9 * C], f16, kind="ExternalInput")
    w2_io = nc.dram_tensor("w2", [C, 9 * C], f16, kind="ExternalInput")
    w3_io = nc.dram_tensor("w3", [C, 9 * C], f16, kind="ExternalInput")
    twT_io = nc.dram_tensor("twT", [C, CI], f16, kind="ExternalInput")
    pwT_io = nc.dram_tensor("pwT", [C, CI], f16, kind="ExternalInput")
    gw_io = nc.dram_tensor("gw", [CI, C], f16, kind="ExternalInput")
    WwT_io = nc.dram_tensor("WwT", [CI, C], f16, kind="ExternalInput")
    xch_io = nc.dram_tensor("xch", [C, CH], f16, kind="ExternalInput")
    ones_io = nc.dram_tensor("ones_row", [1, N], f16, kind="ExternalInput")
    delta_io = nc.dram_tensor("delta_row", [1, CH], f16, kind="ExternalInput")
    out_io = nc.dram_tensor("out", [C, CH], f32, kind="ExternalOutput")

    with tile.TileContext(nc) as tc:
        with tc.tile_pool(name="persist", bufs=1) as pp, \
             tc.tile_pool(name="dram", bufs=1, space="DRAM") as dp:
            zsum = pp.tile([128, MT], f32)
            nb5 = pp.tile([128, 1], f32)
            nc.gpsimd.memset(nb5[:], EXP_BIAS)
            shared = {} if single else {"addr_space": "Shared"}
            zinA = dp.tile([128, HALF], f32)
            zoutA = dp.tile([NCORES, 128, HALF], f32, **shared)
            zinB = dp.tile([128, HALF], f32)
            zoutB = dp.tile([NCORES, 128, HALF], f32, **shared)

            with tc.tile_pool(name="hand", bufs=1) as hp:
                phi16 = hp.tile([CI + 1, N], f16)
                th16 = hp.tile([CI + 1, CH], f16)
                G16 = hp.tile([128, MT * C], f16)
                Gw3 = G16[:].rearrange("p (j c) -> p j c", c=C)
                G8 = hp.tile([128, MT * C], f8)
                G3 = G8[:].rearrange("p (j c) -> p j c", c=C)
                xg16 = hp.tile([C, N], f16)
                xres = hp.tile([C, CH], f16)
                outsb = hp.tile([C, CH], f32)
                s_cache = hp.tile([128, MT * CH], f8e5)
                s3 = s_cache[:].rearrange("p (j n) -> p j n", n=CH)

                def s_sl(j):
                    return s_cache[:, j * CH:(j + 1) * CH]

                def s_pair(jl, o0, w):
                    # [128, 2, w] rhs for a DoubleRow pass-2 pair
                    return s3[:, jl:jl + 2, o0:o0 + w]

                p1ps = None

                def allreduce(zi, zo, jsl):
                    nc.sync.dma_start(zi[:], zsum[:, jsl])
                    if single:
                        for sh in range(NCORES):
                            nc.sync.dma_start(zo[sh], zi[:])
                    else:
                        nc.gpsimd.collective_compute(
                            "AllGather", ALU.bypass,
                            replica_groups=[list(range(NCORES))],
                            ins=[zi.opt()], outs=[zo.opt()])

                def scale_G(hh, zo, eng):
                    # gather shards -> sum -> reciprocal -> scale G half
                    zr = hp.tile([128, NCORES, HALF], f32, tag="zr", name="zr")
                    nc.sync.dma_start(zr[:], zo[:].rearrange("s p h -> p s h"))
                    zf = hp.tile([128, HALF], f32, tag="zf", name="zf",
                                 bufs=2)
                    nc.vector.tensor_reduce(
                        zf[:], zr[:].rearrange("p s h -> p h s"),
                        axis=mybir.AxisListType.X, op=ALU.add)
                    rz = hp.tile([128, HALF], f32, tag="rz", name="rz",
                                 bufs=2)
                    nc.vector.reciprocal(rz[:], zf[:])
                    rz2 = hp.tile([128, HALF], f32, tag="rz2", name="rz2",
                                  bufs=2)
                    nc.vector.tensor_scalar(rz2[:], rz[:], GK, None,
                                            op0=ALU.mult)
                    rzb = rz2[:].unsqueeze(-1).to_broadcast((128, HALF, C))
                    j0 = hh * HALF
                    eng.tensor_mul(
                        G3[:, j0:j0 + HALF, :], Gw3[:, j0:j0 + HALF, :], rzb)

                def pass1_tile(j):
                    fps = p1ps.tile([128, CH], f32, tag="fps", name="fps")
                    for o0, w in SUBS:
                        nc.tensor.matmul(fps[:, o0:o0 + w],
                                         phi16[:, j * 128:(j + 1) * 128],
                                         th16[:, o0:o0 + w],
                                         start=True, stop=True)
                    ssl = s_sl(j)
                    if j % 3 == 2:
                        # Schraudolph on DVE: e5m2 bits = floor(max(fps*S, 0))
                        nc.vector.tensor_scalar(ssl.bitcast(i8), fps[:],
                                                SCH_SCALE, 0.0,
                                                op0=ALU.mult, op1=ALU.max)
                        nc.vector.tensor_reduce(
                            zsum[:, j:j + 1], ssl,
                            axis=mybir.AxisListType.X, op=ALU.add)
                    else:
                        nc.scalar.activation(ssl, fps[:], AF.Exp,
                                             bias=nb5[:], scale=1.0,
                                             accum_out=zsum[:, j:j + 1])

                # ========== PHASE A + PASS 1 (share p1ps PSUM) ==========
                p1ps = tc.alloc_tile_pool(name="p1ps", bufs=2, space="PSUM")
                if True:
                    pm = tc.alloc_tile_pool(name="mid", bufs=1)
                    yh = pm.tile([C, N], f16)
                    yh3 = yh[:].rearrange("c (h w) -> c h w", h=H)

                    # --- A1: convs + upsample ---
                    with tc.tile_pool(name="pa1", bufs=1) as pa, \
                         tc.tile_pool(name="paps1", bufs=2, space="PSUM") as paps:
                        w1sb = pa.tile([C, 9 * C], f16)
                        nc.sync.dma_start(w1sb[:], w1_io[:])
                        w2sb = pa.tile([C, 9 * C], f16)
                        nc.sync.dma_start(w2sb[:], w2_io[:])
                        w3sb = pa.tile([C, 9 * C], f16)
                        nc.sync.dma_start(w3sb[:], w3_io[:])
                        xpad = pa.tile([C, 98, 98], f16)
                        for b in range(4):
                            r0, r1 = 26 * b, min(26 * b + 26, 98)
                            nc.sync.dma_start(xpad[:, r0:r1, :],
                                              xpad_io[:, r0:r1, :])
                        twTsb = hp.tile([C, CI], f16)
                        nc.sync.dma_start(twTsb[:], twT_io[:])
                        pwTsb = hp.tile([C, CI], f16)
                        nc.sync.dma_start(pwTsb[:], pwT_io[:])
                        gwsb = hp.tile([CI, C], f16)
                        nc.sync.dma_start(gwsb[:], gw_io[:])
                        WwTsb = hp.tile([CI, C], f16)
                        nc.sync.dma_start(WwTsb[:], WwT_io[:])
                        xchsb = hp.tile([C, CH], f16)
                        nc.sync.dma_start(xchsb[:], xch_io[:])

                        # conv1: 96x96 -> 48x48, stride 2, pad 1, lrelu(0.2)
                        y1p = pa.tile([C, 50, 50], f16)
                        nc.gpsimd.memset(y1p[:, 0:1, :], 0.0)
                        nc.gpsimd.memset(y1p[:, 49:50, :], 0.0)
                        nc.gpsimd.memset(y1p[:, :, 0:1], 0.0)
                        nc.gpsimd.memset(y1p[:, :, 49:50], 0.0)
                        for g in range(6):
                            ps1 = paps.tile([C, 8, 48], f32, tag="cv", name="ps1")
                            for t in range(9):
                                dy, dx = t // 3, t % 3
                                nc.tensor.matmul(
                                    ps1[:], w1sb[:, t * C:(t + 1) * C],
                                    xpad[:, 16 * g + dy: 16 * g + dy + 16: 2,
                                         dx: dx + 96: 2],
                                    start=(t == 0), stop=(t == 8))
                            ab1 = pa.tile([C, 8 * 48], f32, tag="ab1",
                                          name="ab1")
                            nc.scalar.activation(ab1[:], ps1[:], AF.Abs,
                                                 scale=0.4)
                            nc.vector.scalar_tensor_tensor(
                                y1p[:, 1 + 8 * g: 9 + 8 * g, 1:49], ps1[:], 0.6,
                                ab1[:], op0=ALU.mult, op1=ALU.add)

                        # conv2: 48x48 -> 24x24
                        y2p = pa.tile([C, 26, 26], f16)
                        nc.gpsimd.memset(y2p[:, 0:1, :], 0.0)
                        nc.gpsimd.memset(y2p[:, 25:26, :], 0.0)
                        nc.gpsimd.memset(y2p[:, :, 0:1], 0.0)
                        nc.gpsimd.memset(y2p[:, :, 25:26], 0.0)
                        for g in range(2):
                            ps2 = paps.tile([C, 12, 24], f32, tag="cv", name="ps2")
                            for t in range(9):
                                dy, dx = t // 3, t % 3
                                nc.tensor.matmul(
                                    ps2[:], w2sb[:, t * C:(t + 1) * C],
                                    y1p[:, 24 * g + dy: 24 * g + dy + 24: 2,
                                        dx: dx + 48: 2],
                                    start=(t == 0), stop=(t == 8))
                            ab2 = pa.tile([C, 12 * 24], f32, tag="ab2",
                                          name="ab2")
                            nc.scalar.activation(ab2[:], ps2[:], AF.Abs,
                                                 scale=0.4)
                            nc.vector.scalar_tensor_tensor(
                                y2p[:, 1 + 12 * g: 13 + 12 * g, 1:25], ps2[:], 0.6,
                                ab2[:], op0=ALU.mult, op1=ALU.add)

                        # conv3: 24x24 -> 12x12 (no activation)
                        ps3 = paps.tile([C, 12, 12], f32, tag="cv", name="ps3")
                        for t in range(9):
                            dy, dx = t // 3, t % 3
                            nc.tensor.matmul(
                                ps3[:], w3sb[:, t * C:(t + 1) * C],
                                y2p[:, dy: dy + 24: 2, dx: dx + 24: 2],
                                start=(t == 0), stop=(t == 8))
                        y3v = pa.tile([C, 14, 12], f16)
                        nc.vector.tensor_copy(y3v[:, 1:13, :], ps3[:])
                        nc.vector.tensor_copy(y3v[:, 0:1, :], ps3[:, 0:1, :])
                        nc.vector.tensor_copy(y3v[:, 13:14, :], ps3[:, 11:12, :])

                        # bilinear x8 vertical: out[8k+r] = X + b_r * (Y - X)
                        yvp = pa.tile([C, 96, 14], f16)
                        dv = pa.tile([C, 13, 12], f16)
                        nc.vector.tensor_sub(dv[:], y3v[:, 1:14, :], y3v[:, 0:13, :])
                        for r in range(8):
                            t = (r + 0.5) / 8 - 0.5
                            kr, b = (0, 1 + t) if r < 4 else (1, t)
                            nc.vector.scalar_tensor_tensor(
                                yvp[:, r:96:8, 1:13], dv[:, kr:kr + 12, :], float(b),
                                y3v[:, kr:kr + 12, :], op0=ALU.mult, op1=ALU.add)
                        nc.vector.tensor_copy(yvp[:, :, 0:1], yvp[:, :, 1:2])
                        nc.vector.tensor_copy(yvp[:, :, 13:14], yvp[:, :, 12:13])

                        # bilinear x8 horizontal -> yh [C, 9216] f16.
                        # Row-split: DVE rows 0-47, Pool rows 48-95 so early
                        # sigmoid chunks unblock as soon as DVE's half lands.
                        dh = pa.tile([C, 96, 13], f16)
                        nc.vector.tensor_sub(dh[:], yvp[:, :, 1:14], yvp[:, :, 0:13])
                        for hf in (0, 1):
                            h0 = 48 * hf
                            for r in range(8):
                                t = (r + 0.5) / 8 - 0.5
                                kr, b = (0, 1 + t) if r < 4 else (1, t)
                                nc.vector.scalar_tensor_tensor(
                                    yh3[:, h0:h0 + 48, r:96:8],
                                    dh[:, h0:h0 + 48, kr:kr + 12], float(b),
                                    yvp[:, h0:h0 + 48, kr:kr + 12],
                                    op0=ALU.mult, op1=ALU.add)

                    # --- A2: gate, x_gated, projections ---
                    with tc.tile_pool(name="pa2", bufs=1) as pa, \
                         tc.tile_pool(name="paps2", bufs=1, space="PSUM") as paps:

                        nc.sync.dma_start(th16[CI:CI + 1, :], delta_io[:])
                        nc.sync.dma_start(phi16[CI:CI + 1, :], ones_io[:])

                        # early own-chunk path: sigmoid -> x_gated (residual +
                        # theta source); unblocks pass 1 early
                        yhc = pa.tile([C, CH], f16)
                        with tc.tile_critical():
                            pid = nc.vector.partition_id()
                            col0 = pid * CH
                            nc.vector.tensor_copy(yhc[:], yh[:, bass.ds(col0, CH)])
                        gtc = pa.tile([C, CH], f16)
                        nc.scalar.activation(gtc[:], yhc[:], AF.Sigmoid)
                        nc.gpsimd.tensor_mul(xres[:], gtc[:], xchsb[:])

                        # theta chunk [CI, CH] -> fp8 (with zero companion)
                        for o0, w in SUBS:
                            tps = paps.tile([CI, 512], f32, tag="prj", name="tps",
                                            bufs=2)
                            nc.tensor.matmul(tps[:, 0:w], twTsb[:],
                                             xres[:, o0:o0 + w],
                                             start=True, stop=True)
                            nc.vector.tensor_copy(th16[0:CI, o0:o0 + w],
                                                  tps[:, 0:w])

                        # E^T = gw^T WwT [C, C]
                        eps = paps.tile([C, 512], f32, tag="prj", name="eps",
                                        bufs=2)
                        nc.tensor.matmul(eps[:, 0:C], gwsb[:], WwTsb[:],
                                         start=True, stop=True)
                        ET16 = pa.tile([C, C], f16)
                        nc.vector.tensor_copy(ET16[:], eps[:, 0:C])

                        def gt_group(gg):
                            gps = paps.tile([128, 8 * C], f32, tag="prj",
                                            name="gps", bufs=2)
                            for u in range(8):
                                j = gg * 8 + u
                                nc.tensor.matmul(gps[:, u * C:(u + 1) * C],
                                                 xg16[:, j * 128:(j + 1) * 128],
                                                 ET16[:], start=True, stop=True)
                            nc.vector.tensor_copy(
                                G16[:, gg * 8 * C:(gg + 1) * 8 * C], gps[:])

                        # pipeline over 18 512-col chunks:
                        #   sigmoid -> x_gated -> phi -> fp8; interleave
                        #   G-groups and the first NA pass-1 tiles
                        for i in range(18):
                            sl = slice(i * 512, (i + 1) * 512)
                            xc = hp.tile([C, 512], f16, tag="xc", name="xc",
                                         bufs=4)
                            nc.sync.dma_start(xc[:], x16_io[:, sl])
                            gt = pa.tile([C, 512], f16, tag="gt", name="gt",
                                         bufs=3)
                            nc.scalar.activation(gt[:], yh[:, sl], AF.Sigmoid)
                            nc.vector.tensor_mul(xg16[:, sl], gt[:], xc[:])

                            pps = paps.tile([CI, 512], f32, tag="prj",
                                            name="pps", bufs=2)
                            nc.tensor.matmul(pps[:], pwTsb[:], xg16[:, sl],
                                             start=True, stop=True)
                            nc.vector.tensor_copy(phi16[0:CI, sl], pps[:])

                            if i % 2 == 1:
                                gt_group((i - 1) // 2)
                        for j in range(NA):
                            pass1_tile(j)

                    # ---- PASS 1 main + early pass-2 cols 0-1023 ----
                    pm.release()
                    p2e = tc.alloc_tile_pool(name="p2eps", bufs=1,
                                             space="PSUM")
                    yc01 = [None, None]

                    def pass2_pair01(u, hh, first, last):
                        jl = 2 * u + hh * HALF
                        for ci in (0, 1):
                            nc.tensor.matmul(
                                yc01[ci][:], G3[:, jl:jl + 2, :],
                                s_pair(jl, 512 * ci, 512),
                                start=first, stop=last,
                                perf_mode=PM.DoubleRow, skip_group_check=True)

                    for j in range(NA, MT):
                        pass1_tile(j)
                        if j == NA:
                            allreduce(zinA, zoutA, slice(0, HALF))
                            scale_G(0, zoutA, nc.gpsimd)
                            yc01[0] = p2e.tile([64, 512], f32, name="yc0")
                            yc01[1] = p2e.tile([64, 512], f32, name="yc1")
                        if j >= NA + 2 and j % 2 == 0:
                            u = (j - NA - 2) // 2
                            pass2_pair01(u, 0, first=(u == 0), last=False)
                    pass2_pair01(17, 0, first=False, last=False)
                    allreduce(zinB, zoutB, slice(HALF, MT))
                    scale_G(1, zoutB, nc.vector)
                    for ci in (0, 1):
                        o0 = 512 * ci
                        for u in range(HALF // 2):
                            jl = 2 * u + HALF
                            nc.tensor.matmul(
                                yc01[ci][:], G3[:, jl:jl + 2, :],
                                s_pair(jl, o0, 512),
                                start=False, stop=(u == HALF // 2 - 1),
                                perf_mode=PM.DoubleRow, skip_group_check=True)
                        nc.vector.scalar_tensor_tensor(
                            outsb[:, o0:o0 + 512], yc01[ci][:], 1.0 / GK,
                            xres[:, o0:o0 + 512], op0=ALU.mult, op1=ALU.add)
                        nc.sync.dma_start(out_io[:, o0:o0 + 512],
                                          outsb[:, o0:o0 + 512])
                    p2e.release()
                    p1ps.release()

                    # cols 1024-1151 tail
                    p2l = tc.alloc_tile_pool(name="p2lps", bufs=1,
                                             space="PSUM")
                    yc2 = p2l.tile([64, 128], f32, name="yc2")
                    for hh in (0, 1):
                        for u in range(HALF // 2):
                            jl = 2 * u + hh * HALF
                            nc.tensor.matmul(
                                yc2[:], G3[:, jl:jl + 2, :],
                                s_pair(jl, 1024, 128),
                                start=(hh == 0 and u == 0),
                                stop=(hh == 1 and u == HALF // 2 - 1),
                                perf_mode=PM.DoubleRow, skip_group_check=True)
                    nc.vector.scalar_tensor_tensor(
                        outsb[:, 1024:1152], yc2[:], 1.0 / GK,
                        xres[:, 1024:1152], op0=ALU.mult, op1=ALU.add)
                    nc.sync.dma_start(out_io[:, 1024:1152],
                                      outsb[:, 1024:1152])
                    p2l.release()

    nc.compile()
    return nc


def get_program():
    if "nc" not in _compiled:
        _compiled["nc"] = _build()
    return _compiled["nc"]


def make_in_maps(inputs):
    f16 = np.float16
    x = np.asarray(inputs["x"], np.float32).reshape(C, H, W)
    xflat = np.ascontiguousarray(x.reshape(C, N))
    xpad = np.zeros((C, 98, 98), f16)
    xpad[:, 1:97, 1:97] = x.astype(f16)

    def conv_w(w):
        # [o, i, dy, dx] -> [i, (dy dx), o]
        return np.ascontiguousarray(
            np.asarray(w, np.float32).transpose(1, 2, 3, 0).reshape(C, 9 * C)
        ).astype(f16)

    base = {
        "xpad": xpad,
        "ones_row": np.ones((1, N), f16),
        "delta_row": np.full((1, CH), DELTA, f16),
        "x16": xflat.astype(f16),
        "w1": conv_w(inputs["d1_w"]),
        "w2": conv_w(inputs["d2_w"]),
        "w3": conv_w(inputs["d3_w"]),
        "twT": np.ascontiguousarray(
            np.asarray(inputs["th_w"], np.float32)[:, :, 0, 0].T).astype(f16),
        "pwT": np.ascontiguousarray(
            np.asarray(inputs["ph_w"], np.float32)[:, :, 0, 0].T).astype(f16),
        "gw": np.ascontiguousarray(
            np.asarray(inputs["g_w"], np.float32)[:, :, 0, 0]).astype(f16),
        "WwT": np.ascontiguousarray(
            np.asarray(inputs["W_w"], np.float32)[:, :, 0, 0].T).astype(f16),
    }
    in_maps = []
    for k in range(NCORES):
        m = dict(base)
        m["xch"] = np.ascontiguousarray(
            xflat[:, k * CH:(k + 1) * CH]).astype(f16)
        in_maps.append(m)
    return in_maps


def kernel(**inputs):
    from concourse import bass_utils

    nc = get_program()
    in_maps = make_in_maps(inputs)
    res = bass_utils.run_bass_kernel_spmd(nc, in_maps,
                                          core_ids=list(range(NCORES)))
    out = np.concatenate([res.results[k]["out"] for k in range(NCORES)], axis=1)
    return out.reshape(1, C, H, W).astype(np.float32)


# revision 31
# speedup vs baseline: 1.0187x; 1.0151x over previous
"""Trainium2 Bass kernel for AttentiveNonLocalBlock2D (v2, fp8 DoubleRow).

Per-core SPMD over 8 NeuronCores, sequence-parallel over N=H*W:
  Phase A: 3x stride-2 conv gating unit (fp16 PE, lrelu as one max-STT op)
    -> bilinear x8 upsample (f16, DVE/Pool row split) -> sigmoid gate (ACT)
    -> x_gated f16; projections phi/theta (fp16 PE) quantized to fp8e4;
    G^T = x_gated^T (W_w g_w)^T in fp16 -> fp8e4.
  Pass 1: score tiles f^T[m_tile, n_chunk] via fp8 DoubleRow matmuls
    (zero-companion theta trick: lhsT broadcast, rhs [theta|zeros]);
    exp(f-5) via ACT directly into an fp8e4 SBUF cache; softmax partials
    Z[m] via ACT accum_out / DVE tensor_reduce (AllReduce over cores).
  Pass 2: fp8 DoubleRow over adjacent m-tile pairs, all 72 m-tiles
    accumulated into one PSUM group per 256-col chunk; G is pre-scaled by
    2^GK/Z, final out = psum * 2^-GK + x_gated, DMA per chunk.
"""

import sys

if "/opt/trn_rl_repo" not in sys.path:
    sys.path.insert(0, "/opt/trn_rl_repo")

import numpy as np

NCORES = 8
C, CI, H, W = 64, 32, 96, 96
N = H * W            # 9216
CH = N // NCORES     # 1152 pixels per core
MT = N // 128        # 72 m-tiles
HALF = MT // 2       # 36 (also the s_cacheA tile count)
NA = HALF
SUBS = ((0, 512), (512, 512), (1024, 128))        # 512-col chunks (proj)
SUBS256 = ((0, 256), (256, 256), (512, 256), (768, 256), (1024, 128))
DELTA = 5.5133       # theta row-33 constant: fps = f + DELTA (Schraudolph offset)
EXP_BIAS = -5.0 - DELTA   # ACT tiles: exp(fps + EXP_BIAS) = exp(f - 5)
SCH_SCALE = 5.770780      # 4*log2(e): e5m2 bits = fps * SCH_SCALE (floor, clamp 0)
GK = 256.0           # G pre-scale folded into 1/Z; undone in final STT

_compiled = {}


def _build(single=False):
    import concourse.bacc as bacc
    import concourse.bass as bass
    import concourse.mybir as mybir
    import concourse.tile as tile

    f16 = mybir.dt.float16
    f32 = mybir.dt.float32
    f8 = mybir.dt.float8e4
    f8e5 = mybir.dt.float8e5
    i8 = mybir.dt.int8
    AF = mybir.ActivationFunctionType
    ALU = mybir.AluOpType
    PM = mybir.MatmulPerfMode

    nc = bacc.Bacc("TRN2", target_bir_lowering=False, debug=False,
                   num_devices=1 if single else NCORES)

    xpad_io = nc.dram_tensor("xpad", [C, 98, 98], f16, kind="ExternalInput")
    x16_io = nc.dram_tensor("x16", [C, N], f16, kind="ExternalInput")
    w1_io = nc.dram_tensor("w1", [C, 9 * C], f16, kind="ExternalInput")
    w2_io = nc.dram_tensor("w2", [C, 9 * C], f16, kind="ExternalInput")
    w3_io = nc.dram_tensor("w3", [C, 9 * C], f16, kind="ExternalInput")
    twT_io = nc.dram_tensor("twT", [C, CI], f16, kind="ExternalInput")
    pwT_io = nc.dram_tensor("pwT", [C, CI], f16, kind="ExternalInput")
    gw_io = nc.dram_tensor("gw", [CI, C], f16, kind="ExternalInput")
    WwT_io = nc.dram_tensor("WwT", [CI, C], f16, kind="ExternalInput")
    xch_io = nc.dram_tensor("xch", [C, CH], f16, kind="ExternalInput")
    ones_io = nc.dram_tensor("ones_row", [1, N], f16, kind="ExternalInput")
    delta_io = nc.dram_tensor("delta_row", [1, CH], f16, kind="ExternalInput")
    out_io = nc.dram_tensor("out", [C, CH], f32, kind="ExternalOutput")

    with tile.TileContext(nc) as tc:
        with tc.tile_pool(name="persist", bufs=1) as pp, \
             tc.tile_pool(name="dram", bufs=1, space="DRAM") as dp:
            zsum = pp.tile([128, MT], f32)
            nb5 = pp.tile([128, 1], f32)
            nc.gpsimd.memset(nb5[:], EXP_BIAS)
            shared = {} if single else {"addr_space": "Shared"}
            zinA = dp.tile([128, HALF], f32)
            zoutA = dp.tile([NCORES, 128, HALF], f32, **shared)
            zinB = dp.tile([128, HALF], f32)
            zoutB = dp.tile([NCORES, 128, HALF], f32, **shared)

            with tc.tile_pool(name="hand", bufs=1) as hp:
                phi16 = hp.tile([CI + 1, N], f16)
                th16 = hp.tile([CI + 1, CH], f16)
                G16 = hp.tile([128, MT * C], f16)
                Gw3 = G16[:].rearrange("p (j c) -> p j c", c=C)
                G8 = hp.tile([128, MT * C], f8)
                G3 = G8[:].rearrange("p (j c) -> p j c", c=C)
                xg16 = hp.tile([C, N], f16)
                xres = hp.tile([C, CH], f16)
                outsb = hp.tile([C, CH], f32)
                s_cache = hp.tile([128, MT * CH], f8e5)
                s3 = s_cache[:].rearrange("p (j n) -> p j n", n=CH)

                def s_sl(j):
                    return s_cache[:, j * CH:(j + 1) * CH]

                def s_pair(jl, o0, w):
                    # [128, 2, w] rhs for a DoubleRow pass-2 pair
                    return s3[:, jl:jl + 2, o0:o0 + w]

                p1ps = None

                def allreduce(zi, zo, jsl):
                    nc.sync.dma_start(zi[:], zsum[:, jsl])
                    if single:
                        for sh in range(NCORES):
                            nc.sync.dma_start(zo[sh], zi[:])
                    else:
                        nc.gpsimd.collective_compute(
                            "AllGather", ALU.bypass,
                            replica_groups=[list(range(NCORES))],
                            ins=[zi.opt()], outs=[zo.opt()])

                def scale_G(hh, zo, eng):
                    # gather shards -> sum -> reciprocal -> scale G half
                    zr = hp.tile([128, NCORES, HALF], f32, tag="zr", name="zr")
                    nc.sync.dma_start(zr[:], zo[:].rearrange("s p h -> p s h"))
                    zf = hp.tile([128, HALF], f32, tag="zf", name="zf",
                                 bufs=2)
                    nc.vector.tensor_reduce(
                        zf[:], zr[:].rearrange("p s h -> p h s"),
                        axis=mybir.AxisListType.X, op=ALU.add)
                    rz = hp.tile([128, HALF], f32, tag="rz", name="rz",
                                 bufs=2)
                    nc.vector.reciprocal(rz[:], zf[:])
                    rz2 = hp.tile([128, HALF], f32, tag="rz2", name="rz2",
                                  bufs=2)
                    nc.vector.tensor_scalar(rz2[:], rz[:], GK, None,
                                            op0=ALU.mult)
                    rzb = rz2[:].unsqueeze(-1).to_broadcast((128, HALF, C))
                    j0 = hh * HALF
                    eng.tensor_mul(
                        G3[:, j0:j0 + HALF, :], Gw3[:, j0:j0 + HALF, :], rzb)

                def pass1_tile(j):
                    fps = p1ps.tile([128, CH], f32, tag="fps", name="fps")
                    for o0, w in SUBS:
                        nc.tensor.matmul(fps[:, o0:o0 + w],
                                         phi16[:, j * 128:(j + 1) * 128],
                                         th16[:, o0:o0 + w],
                                         start=True, stop=True)
                    ssl = s_sl(j)
                    if j % 3 == 2:
                        # Schraudolph on DVE: e5m2 bits = floor(max(fps*S, 0))
                        nc.vector.tensor_scalar(ssl.bitcast(i8), fps[:],
                                                SCH_SCALE, 0.0,
                                                op0=ALU.mult, op1=ALU.max)
                        nc.vector.tensor_reduce(
                            zsum[:, j:j + 1], ssl,
                            axis=mybir.AxisListType.X, op=ALU.add)
                    else:
                        nc.scalar.activation(ssl, fps[:], AF.Exp,
                                             bias=nb5[:], scale=1.0,
                                             accum_out=zsum[:, j:j + 1])

                # ========== PHASE A + PASS 1 (share p1ps PSUM) ==========
                p1ps = tc.alloc_tile_pool(name="p1ps", bufs=2, space="PSUM")
                if True:
                    pm = tc.alloc_tile_pool(name="mid", bufs=1)
                    yh = pm.tile([C, N], f16)
                    yh3 = yh[:].rearrange("c (h w) -> c h w", h=H)

                    # --- A1: convs + upsample ---
                    with tc.tile_pool(name="pa1", bufs=1) as pa, \
                         tc.tile_pool(name="paps1", bufs=2, space="PSUM") as paps:
                        w1sb = pa.tile([C, 9 * C], f16)
                        nc.sync.dma_start(w1sb[:], w1_io[:])
                        w2sb = pa.tile([C, 9 * C], f16)
                        nc.sync.dma_start(w2sb[:], w2_io[:])
                        w3sb = pa.tile([C, 9 * C], f16)
                        nc.sync.dma_start(w3sb[:], w3_io[:])
                        xpad = pa.tile([C, 98, 98], f16)
                        for b in range(4):
                            r0, r1 = 26 * b, min(26 * b + 26, 98)
                            nc.sync.dma_start(xpad[:, r0:r1, :],
                                              xpad_io[:, r0:r1, :])
                        twTsb = hp.tile([C, CI], f16)
                        nc.sync.dma_start(twTsb[:], twT_io[:])
                        pwTsb = hp.tile([C, CI], f16)
                        nc.sync.dma_start(pwTsb[:], pwT_io[:])
                        gwsb = hp.tile([CI, C], f16)
                        nc.sync.dma_start(gwsb[:], gw_io[:])
                        WwTsb = hp.tile([CI, C], f16)
                        nc.sync.dma_start(WwTsb[:], WwT_io[:])
                        xchsb = hp.tile([C, CH], f16)
                        nc.sync.dma_start(xchsb[:], xch_io[:])

                        # conv1: 96x96 -> 48x48, stride 2, pad 1, lrelu(0.2)
                        y1p = pa.tile([C, 50, 50], f16)
                        nc.gpsimd.memset(y1p[:, 0:1, :], 0.0)
                        nc.gpsimd.memset(y1p[:, 49:50, :], 0.0)
                        nc.gpsimd.memset(y1p[:, :, 0:1], 0.0)
                        nc.gpsimd.memset(y1p[:, :, 49:50], 0.0)
                        for g in range(6):
                            ps1 = paps.tile([C, 8, 48], f32, tag="cv", name="ps1")
                            for t in range(9):
                                dy, dx = t // 3, t % 3
                                nc.tensor.matmul(
                                    ps1[:], w1sb[:, t * C:(t + 1) * C],
                                    xpad[:, 16 * g + dy: 16 * g + dy + 16: 2,
                                         dx: dx + 96: 2],
                                    start=(t == 0), stop=(t == 8))
                            ab1 = pa.tile([C, 8 * 48], f32, tag="ab1",
                                          name="ab1")
                            nc.scalar.activation(ab1[:], ps1[:], AF.Abs,
                                                 scale=0.4)
                            nc.vector.scalar_tensor_tensor(
                                y1p[:, 1 + 8 * g: 9 + 8 * g, 1:49], ps1[:], 0.6,
                                ab1[:], op0=ALU.mult, op1=ALU.add)

                        # conv2: 48x48 -> 24x24
                        y2p = pa.tile([C, 26, 26], f16)
                        nc.gpsimd.memset(y2p[:, 0:1, :], 0.0)
                        nc.gpsimd.memset(y2p[:, 25:26, :], 0.0)
                        nc.gpsimd.memset(y2p[:, :, 0:1], 0.0)
                        nc.gpsimd.memset(y2p[:, :, 25:26], 0.0)
                        for g in range(2):
                            ps2 = paps.tile([C, 12, 24], f32, tag="cv", name="ps2")
                            for t in range(9):
                                dy, dx = t // 3, t % 3
                                nc.tensor.matmul(
                                    ps2[:], w2sb[:, t * C:(t + 1) * C],
                                    y1p[:, 24 * g + dy: 24 * g + dy + 24: 2,
                                        dx: dx + 48: 2],
                                    start=(t == 0), stop=(t == 8))
                            ab2 = pa.tile([C, 12 * 24], f32, tag="ab2",
                                          name="ab2")
                            nc.scalar.activation(ab2[:], ps2[:], AF.Abs,
                                                 scale=0.4)
                            nc.vector.scalar_tensor_tensor(
                                y2p[:, 1 + 12 * g: 13 + 12 * g, 1:25], ps2[:], 0.6,
                                ab2[:], op0=ALU.mult, op1=ALU.add)

                        # conv3: 24x24 -> 12x12 (no activation)
                        ps3 = paps.tile([C, 12, 12], f32, tag="cv", name="ps3")
                        for t in range(9):
                            dy, dx = t // 3, t % 3
                            nc.tensor.matmul(
                                ps3[:], w3sb[:, t * C:(t + 1) * C],
                                y2p[:, dy: dy + 24: 2, dx: dx + 24: 2],
                                start=(t == 0), stop=(t == 8))
                        y3v = pa.tile([C, 14, 12], f16)
                        nc.vector.tensor_copy(y3v[:, 1:13, :], ps3[:])
                        nc.vector.tensor_copy(y3v[:, 0:1, :], ps3[:, 0:1, :])
                        nc.vector.tensor_copy(y3v[:, 13:14, :], ps3[:, 11:12, :])

                        # bilinear x8 vertical: out[8k+r] = X + b_r * (Y - X)
                        yvp = pa.tile([C, 96, 14], f16)
                        dv = pa.tile([C, 13, 12], f16)
                        nc.vector.tensor_sub(dv[:], y3v[:, 1:14, :], y3v[:, 0:13, :])
                        for r in range(8):
                            t = (r + 0.5) / 8 - 0.5
                            kr, b = (0, 1 + t) if r < 4 else (1, t)
                            nc.vector.scalar_tensor_tensor(
                                yvp[:, r:96:8, 1:13], dv[:, kr:kr + 12, :], float(b),
                                y3v[:, kr:kr + 12, :], op0=ALU.mult, op1=ALU.add)
                        nc.vector.tensor_copy(yvp[:, :, 0:1], yvp[:, :, 1:2])
                        nc.vector.tensor_copy(yvp[:, :, 13:14], yvp[:, :, 12:13])

                        # bilinear x8 horizontal -> yh [C, 9216] f16.
                        # Row-split: DVE rows 0-47, Pool rows 48-95 so early
                        # sigmoid chunks unblock as soon as DVE's half lands.
                        dh = pa.tile([C, 96, 13], f16)
                        nc.vector.tensor_sub(dh[:], yvp[:, :, 1:14], yvp[:, :, 0:13])
                        for hf in (0, 1):
                            h0 = 48 * hf
                            for r in range(8):
                                t = (r + 0.5) / 8 - 0.5
                                kr, b = (0, 1 + t) if r < 4 else (1, t)
                                if r % 4 == 3:
                                    dhs_t = pa.tile([C, 48, 12], f16,
                                                    tag="dhs", name="dhs",
                                                    bufs=2)
                                    nc.vector.tensor_scalar(
                                        dhs_t[:], dh[:, h0:h0 + 48, kr:kr + 12],
                                        float(b), None, op0=ALU.mult)
                                    nc.gpsimd.tensor_add(
                                        yh3[:, h0:h0 + 48, r:96:8], dhs_t[:],
                                        yvp[:, h0:h0 + 48, kr:kr + 12])
                                else:
                                    nc.vector.scalar_tensor_tensor(
                                        yh3[:, h0:h0 + 48, r:96:8],
                                        dh[:, h0:h0 + 48, kr:kr + 12], float(b),
                                        yvp[:, h0:h0 + 48, kr:kr + 12],
                                        op0=ALU.mult, op1=ALU.add)

                    # --- A2: gate, x_gated, projections ---
                    with tc.tile_pool(name="pa2", bufs=1) as pa, \
                         tc.tile_pool(name="paps2", bufs=1, space="PSUM") as paps:

                        nc.sync.dma_start(th16[CI:CI + 1, :], delta_io[:])
                        nc.sync.dma_start(phi16[CI:CI + 1, :], ones_io[:])

                        # early own-chunk path: sigmoid -> x_gated (residual +
                        # theta source); unblocks pass 1 early
                        yhc = pa.tile([C, CH], f16)
                        with tc.tile_critical():
                            pid = nc.vector.partition_id()
                            col0 = pid * CH
                            nc.vector.tensor_copy(yhc[:], yh[:, bass.ds(col0, CH)])
                        gtc = pa.tile([C, CH], f16)
                        nc.scalar.activation(gtc[:], yhc[:], AF.Sigmoid)
                        nc.gpsimd.tensor_mul(xres[:], gtc[:], xchsb[:])

                        # theta chunk [CI, CH] -> fp8 (with zero companion)
                        for o0, w in SUBS:
                            tps = paps.tile([CI, 512], f32, tag="prj", name="tps",
                                            bufs=2)
                            nc.tensor.matmul(tps[:, 0:w], twTsb[:],
                                             xres[:, o0:o0 + w],
                                             start=True, stop=True)
                            nc.vector.tensor_copy(th16[0:CI, o0:o0 + w],
                                                  tps[:, 0:w])

                        # E^T = gw^T WwT [C, C]
                        eps = paps.tile([C, 512], f32, tag="prj", name="eps",
                                        bufs=2)
                        nc.tensor.matmul(eps[:, 0:C], gwsb[:], WwTsb[:],
                                         start=True, stop=True)
                        ET16 = pa.tile([C, C], f16)
                        nc.vector.tensor_copy(ET16[:], eps[:, 0:C])

                        def gt_group(gg):
                            gps = paps.tile([128, 8 * C], f32, tag="prj",
                                            name="gps", bufs=2)
                            for u in range(8):
                                j = gg * 8 + u
                                nc.tensor.matmul(gps[:, u * C:(u + 1) * C],
                                                 xg16[:, j * 128:(j + 1) * 128],
                                                 ET16[:], start=True, stop=True)
                            nc.scalar.activation(
                                G16[:, gg * 8 * C:(gg + 1) * 8 * C], gps[:],
                                AF.Copy)

                        # pipeline over 18 512-col chunks:
                        #   sigmoid -> x_gated -> phi -> fp8; interleave
                        #   G-groups and the first NA pass-1 tiles
                        for i in range(18):
                            sl = slice(i * 512, (i + 1) * 512)
                            xc = hp.tile([C, 512], f16, tag="xc", name="xc",
                                         bufs=4)
                            nc.sync.dma_start(xc[:], x16_io[:, sl])
                            gt = pa.tile([C, 512], f16, tag="gt", name="gt",
                                         bufs=3)
                            nc.scalar.activation(gt[:], yh[:, sl], AF.Sigmoid)
                            nc.vector.tensor_mul(xg16[:, sl], gt[:], xc[:])

                            pps = paps.tile([CI, 512], f32, tag="prj",
                                            name="pps", bufs=2)
                            nc.tensor.matmul(pps[:], pwTsb[:], xg16[:, sl],
                                             start=True, stop=True)
                            nc.vector.tensor_copy(phi16[0:CI, sl], pps[:])

                            if i % 2 == 1:
                                gt_group((i - 1) // 2)
                        for j in range(NA):
                            pass1_tile(j)

                    # ---- PASS 1 main + early pass-2 cols 0-1023 ----
                    pm.release()
                    p2e = tc.alloc_tile_pool(name="p2eps", bufs=1,
                                             space="PSUM")
                    yc01 = [None, None]

                    def pass2_pair01(u, hh, first, last):
                        jl = 2 * u + hh * HALF
                        for ci in (0, 1):
                            nc.tensor.matmul(
                                yc01[ci][:], G3[:, jl:jl + 2, :],
                                s_pair(jl, 512 * ci, 512),
                                start=first, stop=last,
                                perf_mode=PM.DoubleRow, skip_group_check=True)

                    for j in range(NA, MT):
                        pass1_tile(j)
                        if j == NA:
                            allreduce(zinA, zoutA, slice(0, HALF))
                            scale_G(0, zoutA, nc.gpsimd)
                            yc01[0] = p2e.tile([64, 512], f32, name="yc0")
                            yc01[1] = p2e.tile([64, 512], f32, name="yc1")
                        if j >= NA + 2 and j % 2 == 0:
                            u = (j - NA - 2) // 2
                            pass2_pair01(u, 0, first=(u == 0), last=False)
                    pass2_pair01(17, 0, first=False, last=False)
                    allreduce(zinB, zoutB, slice(HALF, MT))
                    scale_G(1, zoutB, nc.vector)
                    for ci in (0, 1):
                        o0 = 512 * ci
                        for u in range(HALF // 2):
                            jl = 2 * u + HALF
                            nc.tensor.matmul(
                                yc01[ci][:], G3[:, jl:jl + 2, :],
                                s_pair(jl, o0, 512),
                                start=False, stop=(u == HALF // 2 - 1),
                                perf_mode=PM.DoubleRow, skip_group_check=True)
                        nc.vector.scalar_tensor_tensor(
                            outsb[:, o0:o0 + 512], yc01[ci][:], 1.0 / GK,
                            xres[:, o0:o0 + 512], op0=ALU.mult, op1=ALU.add)
                        nc.sync.dma_start(out_io[:, o0:o0 + 512],
                                          outsb[:, o0:o0 + 512])
                    p2e.release()
                    p1ps.release()

                    # cols 1024-1151 tail
                    p2l = tc.alloc_tile_pool(name="p2lps", bufs=1,
                                             space="PSUM")
                    yc2 = p2l.tile([64, 128], f32, name="yc2")
                    for hh in (0, 1):
                        for u in range(HALF // 2):
                            jl = 2 * u + hh * HALF
                            nc.tensor.matmul(
                                yc2[:], G3[:, jl:jl + 2, :],
                                s_pair(jl, 1024, 128),
                                start=(hh == 0 and u == 0),
                                stop=(hh == 1 and u == HALF // 2 - 1),
                                perf_mode=PM.DoubleRow, skip_group_check=True)
                    nc.vector.scalar_tensor_tensor(
                        outsb[:, 1024:1152], yc2[:], 1.0 / GK,
                        xres[:, 1024:1152], op0=ALU.mult, op1=ALU.add)
                    nc.sync.dma_start(out_io[:, 1024:1152],
                                      outsb[:, 1024:1152])
                    p2l.release()

    nc.compile()
    return nc


def get_program():
    if "nc" not in _compiled:
        _compiled["nc"] = _build()
    return _compiled["nc"]


def make_in_maps(inputs):
    f16 = np.float16
    x = np.asarray(inputs["x"], np.float32).reshape(C, H, W)
    xflat = np.ascontiguousarray(x.reshape(C, N))
    xpad = np.zeros((C, 98, 98), f16)
    xpad[:, 1:97, 1:97] = x.astype(f16)

    def conv_w(w):
        # [o, i, dy, dx] -> [i, (dy dx), o]
        return np.ascontiguousarray(
            np.asarray(w, np.float32).transpose(1, 2, 3, 0).reshape(C, 9 * C)
        ).astype(f16)

    base = {
        "xpad": xpad,
        "ones_row": np.ones((1, N), f16),
        "delta_row": np.full((1, CH), DELTA, f16),
        "x16": xflat.astype(f16),
        "w1": conv_w(inputs["d1_w"]),
        "w2": conv_w(inputs["d2_w"]),
        "w3": conv_w(inputs["d3_w"]),
        "twT": np.ascontiguousarray(
            np.asarray(inputs["th_w"], np.float32)[:, :, 0, 0].T).astype(f16),
        "pwT": np.ascontiguousarray(
            np.asarray(inputs["ph_w"], np.float32)[:, :, 0, 0].T).astype(f16),
        "gw": np.ascontiguousarray(
            np.asarray(inputs["g_w"], np.float32)[:, :, 0, 0]).astype(f16),
        "WwT": np.ascontiguousarray(
            np.asarray(inputs["W_w"], np.float32)[:, :, 0, 0].T).astype(f16),
    }
    in_maps = []
    for k in range(NCORES):
        m = dict(base)
        m["xch"] = np.ascontiguousarray(
            xflat[:, k * CH:(k + 1) * CH]).astype(f16)
        in_maps.append(m)
    return in_maps


def kernel(**inputs):
    from concourse import bass_utils

    nc = get_program()
    in_maps = make_in_maps(inputs)
    res = bass_utils.run_bass_kernel_spmd(nc, in_maps,
                                          core_ids=list(range(NCORES)))
    out = np.concatenate([res.results[k]["out"] for k in range(NCORES)], axis=1)
    return out.reshape(1, C, H, W).astype(np.float32)


# revision 32
# speedup vs baseline: 1.0285x; 1.0095x over previous
"""Trainium2 Bass kernel for AttentiveNonLocalBlock2D (v2, fp8 DoubleRow).

Per-core SPMD over 8 NeuronCores, sequence-parallel over N=H*W:
  Phase A: 3x stride-2 conv gating unit (fp16 PE, lrelu as one max-STT op)
    -> bilinear x8 upsample (f16, DVE/Pool row split) -> sigmoid gate (ACT)
    -> x_gated f16; projections phi/theta (fp16 PE) quantized to fp8e4;
    G^T = x_gated^T (W_w g_w)^T in fp16 -> fp8e4.
  Pass 1: score tiles f^T[m_tile, n_chunk] via fp8 DoubleRow matmuls
    (zero-companion theta trick: lhsT broadcast, rhs [theta|zeros]);
    exp(f-5) via ACT directly into an fp8e4 SBUF cache; softmax partials
    Z[m] via ACT accum_out / DVE tensor_reduce (AllReduce over cores).
  Pass 2: fp8 DoubleRow over adjacent m-tile pairs, all 72 m-tiles
    accumulated into one PSUM group per 256-col chunk; G is pre-scaled by
    2^GK/Z, final out = psum * 2^-GK + x_gated, DMA per chunk.
"""

import sys

if "/opt/trn_rl_repo" not in sys.path:
    sys.path.insert(0, "/opt/trn_rl_repo")

import numpy as np

NCORES = 8
C, CI, H, W = 64, 32, 96, 96
N = H * W            # 9216
CH = N // NCORES     # 1152 pixels per core
MT = N // 128        # 72 m-tiles
HALF = MT // 2       # 36 (also the s_cacheA tile count)
NA = HALF
SUBS = ((0, 512), (512, 512), (1024, 128))        # 512-col chunks (proj)
SUBS256 = ((0, 256), (256, 256), (512, 256), (768, 256), (1024, 128))
DELTA = 5.5133       # theta row-33 constant: fps = f + DELTA (Schraudolph offset)
EXP_BIAS = -5.0 - DELTA   # ACT tiles: exp(fps + EXP_BIAS) = exp(f - 5)
SCH_SCALE = 5.770780      # 4*log2(e): e5m2 bits = fps * SCH_SCALE (floor, clamp 0)
GK = 256.0           # G pre-scale folded into 1/Z; undone in final STT

_compiled = {}


def _build(single=False):
    import concourse.bacc as bacc
    import concourse.bass as bass
    import concourse.mybir as mybir
    import concourse.tile as tile

    f16 = mybir.dt.float16
    f32 = mybir.dt.float32
    f8 = mybir.dt.float8e4
    f8e5 = mybir.dt.float8e5
    i8 = mybir.dt.int8
    AF = mybir.ActivationFunctionType
    ALU = mybir.AluOpType
    PM = mybir.MatmulPerfMode

    nc = bacc.Bacc("TRN2", target_bir_lowering=False, debug=False,
                   num_devices=1 if single else NCORES)

    xpad_io = nc.dram_tensor("xpad", [C, 98, 98], f16, kind="ExternalInput")
    x16_io = nc.dram_tensor("x16", [C, N], f16, kind="ExternalInput")
    w1_io = nc.dram_tensor("w1", [C, 9 * C], f16, kind="ExternalInput")
    w2_io = nc.dram_tensor("w2", [C, 9 * C], f16, kind="ExternalInput")
    w3_io = nc.dram_tensor("w3", [C, 9 * C], f16, kind="ExternalInput")
    twT_io = nc.dram_tensor("twT", [C, CI], f16, kind="ExternalInput")
    pwT_io = nc.dram_tensor("pwT", [C, CI], f16, kind="ExternalInput")
    gw_io = nc.dram_tensor("gw", [CI, C], f16, kind="ExternalInput")
    WwT_io = nc.dram_tensor("WwT", [CI, C], f16, kind="ExternalInput")
    xch_io = nc.dram_tensor("xch", [C, CH], f16, kind="ExternalInput")
    ones_io = nc.dram_tensor("ones_row", [1, N], f16, kind="ExternalInput")
    delta_io = nc.dram_tensor("delta_row", [1, CH], f16, kind="ExternalInput")
    out_io = nc.dram_tensor("out", [C, CH], f32, kind="ExternalOutput")

    with tile.TileContext(nc) as tc:
        with tc.tile_pool(name="persist", bufs=1) as pp, \
             tc.tile_pool(name="dram", bufs=1, space="DRAM") as dp:
            zsum = pp.tile([128, MT], f32)
            nb5 = pp.tile([128, 1], f32)
            nc.gpsimd.memset(nb5[:], EXP_BIAS)
            shared = {} if single else {"addr_space": "Shared"}
            zinA = dp.tile([128, HALF], f32)
            zoutA = dp.tile([NCORES, 128, HALF], f32, **shared)
            zinB = dp.tile([128, HALF], f32)
            zoutB = dp.tile([NCORES, 128, HALF], f32, **shared)

            with tc.tile_pool(name="hand", bufs=1) as hp:
                phi16 = hp.tile([CI + 1, N], f16)
                th16 = hp.tile([CI + 1, CH], f16)
                G16 = hp.tile([128, MT * C], f16)
                Gw3 = G16[:].rearrange("p (j c) -> p j c", c=C)
                G8 = hp.tile([128, MT * C], f8)
                G3 = G8[:].rearrange("p (j c) -> p j c", c=C)
                xg16 = hp.tile([C, N], f16)
                xres = hp.tile([C, CH], f16)
                outsb = hp.tile([C, CH], f32)
                s_cache = hp.tile([128, MT * CH], f8e5)
                s3 = s_cache[:].rearrange("p (j n) -> p j n", n=CH)

                def s_sl(j):
                    return s_cache[:, j * CH:(j + 1) * CH]

                def s_pair(jl, o0, w):
                    # [128, 2, w] rhs for a DoubleRow pass-2 pair
                    return s3[:, jl:jl + 2, o0:o0 + w]

                p1ps = None

                def allreduce(zi, zo, jsl):
                    nc.sync.dma_start(zi[:], zsum[:, jsl])
                    if single:
                        for sh in range(NCORES):
                            nc.sync.dma_start(zo[sh], zi[:])
                    else:
                        nc.gpsimd.collective_compute(
                            "AllGather", ALU.bypass,
                            replica_groups=[list(range(NCORES))],
                            ins=[zi.opt()], outs=[zo.opt()])

                def scale_G(hh, zo, eng):
                    # gather shards -> sum -> reciprocal -> scale G half
                    zr = hp.tile([128, NCORES, HALF], f32, tag="zr", name="zr")
                    nc.sync.dma_start(zr[:], zo[:].rearrange("s p h -> p s h"))
                    zf = hp.tile([128, HALF], f32, tag="zf", name="zf",
                                 bufs=2)
                    nc.vector.tensor_reduce(
                        zf[:], zr[:].rearrange("p s h -> p h s"),
                        axis=mybir.AxisListType.X, op=ALU.add)
                    rz = hp.tile([128, HALF], f32, tag="rz", name="rz",
                                 bufs=2)
                    nc.vector.reciprocal(rz[:], zf[:])
                    rz2 = hp.tile([128, HALF], f32, tag="rz2", name="rz2",
                                  bufs=2)
                    nc.vector.tensor_scalar(rz2[:], rz[:], GK, None,
                                            op0=ALU.mult)
                    rzb = rz2[:].unsqueeze(-1).to_broadcast((128, HALF, C))
                    j0 = hh * HALF
                    eng.tensor_mul(
                        G3[:, j0:j0 + HALF, :], Gw3[:, j0:j0 + HALF, :], rzb)

                def pass1_tile(j):
                    fps = p1ps.tile([128, CH], f32, tag="fps", name="fps")
                    for o0, w in SUBS:
                        nc.tensor.matmul(fps[:, o0:o0 + w],
                                         phi16[:, j * 128:(j + 1) * 128],
                                         th16[:, o0:o0 + w],
                                         start=True, stop=True)
                    ssl = s_sl(j)
                    if j % 3 == 2:
                        # Schraudolph on DVE: e5m2 bits = floor(max(fps*S, 0))
                        nc.vector.tensor_scalar(ssl.bitcast(i8), fps[:],
                                                SCH_SCALE, 0.0,
                                                op0=ALU.mult, op1=ALU.max)
                        nc.vector.tensor_reduce(
                            zsum[:, j:j + 1], ssl,
                            axis=mybir.AxisListType.X, op=ALU.add)
                    else:
                        nc.scalar.activation(ssl, fps[:], AF.Exp,
                                             bias=nb5[:], scale=1.0,
                                             accum_out=zsum[:, j:j + 1])

                # ========== PHASE A + PASS 1 (share p1ps PSUM) ==========
                p1ps = tc.alloc_tile_pool(name="p1ps", bufs=2, space="PSUM")
                if True:
                    pm = tc.alloc_tile_pool(name="mid", bufs=1)
                    yh = pm.tile([C, N], f16)
                    yh3 = yh[:].rearrange("c (h w) -> c h w", h=H)

                    # --- A1: convs + upsample ---
                    with tc.tile_pool(name="pa1", bufs=1) as pa, \
                         tc.tile_pool(name="paps1", bufs=2, space="PSUM") as paps:
                        w1sb = pa.tile([C, 9 * C], f16)
                        nc.sync.dma_start(w1sb[:], w1_io[:])
                        w2sb = pa.tile([C, 9 * C], f16)
                        nc.sync.dma_start(w2sb[:], w2_io[:])
                        w3sb = pa.tile([C, 9 * C], f16)
                        nc.sync.dma_start(w3sb[:], w3_io[:])
                        xpad = pa.tile([C, 98, 98], f16)
                        for b in range(4):
                            r0, r1 = 26 * b, min(26 * b + 26, 98)
                            nc.sync.dma_start(xpad[:, r0:r1, :],
                                              xpad_io[:, r0:r1, :])
                        twTsb = hp.tile([C, CI], f16)
                        nc.sync.dma_start(twTsb[:], twT_io[:])
                        pwTsb = hp.tile([C, CI], f16)
                        nc.sync.dma_start(pwTsb[:], pwT_io[:])
                        gwsb = hp.tile([CI, C], f16)
                        nc.sync.dma_start(gwsb[:], gw_io[:])
                        WwTsb = hp.tile([CI, C], f16)
                        nc.sync.dma_start(WwTsb[:], WwT_io[:])
                        xchsb = hp.tile([C, CH], f16)
                        nc.sync.dma_start(xchsb[:], xch_io[:])

                        # conv1: 96x96 -> 48x48, stride 2, pad 1, lrelu(0.2)
                        y1p = pa.tile([C, 50, 50], f16)
                        nc.gpsimd.memset(y1p[:, 0:1, :], 0.0)
                        nc.gpsimd.memset(y1p[:, 49:50, :], 0.0)
                        nc.gpsimd.memset(y1p[:, :, 0:1], 0.0)
                        nc.gpsimd.memset(y1p[:, :, 49:50], 0.0)
                        for g in range(6):
                            ps1 = paps.tile([C, 8, 48], f32, tag="cv", name="ps1")
                            for t in range(9):
                                dy, dx = t // 3, t % 3
                                nc.tensor.matmul(
                                    ps1[:], w1sb[:, t * C:(t + 1) * C],
                                    xpad[:, 16 * g + dy: 16 * g + dy + 16: 2,
                                         dx: dx + 96: 2],
                                    start=(t == 0), stop=(t == 8))
                            ab1 = pa.tile([C, 8 * 48], f32, tag="ab1",
                                          name="ab1")
                            nc.scalar.activation(ab1[:], ps1[:], AF.Abs,
                                                 scale=0.4)
                            nc.vector.scalar_tensor_tensor(
                                y1p[:, 1 + 8 * g: 9 + 8 * g, 1:49], ps1[:], 0.6,
                                ab1[:], op0=ALU.mult, op1=ALU.add)

                        # conv2: 48x48 -> 24x24
                        y2p = pa.tile([C, 26, 26], f16)
                        nc.gpsimd.memset(y2p[:, 0:1, :], 0.0)
                        nc.gpsimd.memset(y2p[:, 25:26, :], 0.0)
                        nc.gpsimd.memset(y2p[:, :, 0:1], 0.0)
                        nc.gpsimd.memset(y2p[:, :, 25:26], 0.0)
                        for g in range(2):
                            ps2 = paps.tile([C, 12, 24], f32, tag="cv", name="ps2")
                            for t in range(9):
                                dy, dx = t // 3, t % 3
                                nc.tensor.matmul(
                                    ps2[:], w2sb[:, t * C:(t + 1) * C],
                                    y1p[:, 24 * g + dy: 24 * g + dy + 24: 2,
                                        dx: dx + 48: 2],
                                    start=(t == 0), stop=(t == 8))
                            ab2 = pa.tile([C, 12 * 24], f32, tag="ab2",
                                          name="ab2")
                            nc.scalar.activation(ab2[:], ps2[:], AF.Abs,
                                                 scale=0.4)
                            nc.vector.scalar_tensor_tensor(
                                y2p[:, 1 + 12 * g: 13 + 12 * g, 1:25], ps2[:], 0.6,
                                ab2[:], op0=ALU.mult, op1=ALU.add)

                        # conv3: 24x24 -> 12x12 (no activation)
                        ps3 = paps.tile([C, 12, 12], f32, tag="cv", name="ps3")
                        for t in range(9):
                            dy, dx = t // 3, t % 3
                            nc.tensor.matmul(
                                ps3[:], w3sb[:, t * C:(t + 1) * C],
                                y2p[:, dy: dy + 24: 2, dx: dx + 24: 2],
                                start=(t == 0), stop=(t == 8))
                        y3v = pa.tile([C, 14, 12], f16)
                        nc.vector.tensor_copy(y3v[:, 1:13, :], ps3[:])
                        nc.vector.tensor_copy(y3v[:, 0:1, :], ps3[:, 0:1, :])
                        nc.vector.tensor_copy(y3v[:, 13:14, :], ps3[:, 11:12, :])

                        # bilinear x8 vertical: out[8k+r] = X + b_r * (Y - X)
                        yvp = pa.tile([C, 96, 14], f16)
                        dv = pa.tile([C, 13, 12], f16)
                        nc.vector.tensor_sub(dv[:], y3v[:, 1:14, :], y3v[:, 0:13, :])
                        for r in range(8):
                            t = (r + 0.5) / 8 - 0.5
                            kr, b = (0, 1 + t) if r < 4 else (1, t)
                            nc.vector.scalar_tensor_tensor(
                                yvp[:, r:96:8, 1:13], dv[:, kr:kr + 12, :], float(b),
                                y3v[:, kr:kr + 12, :], op0=ALU.mult, op1=ALU.add)
                        nc.vector.tensor_copy(yvp[:, :, 0:1], yvp[:, :, 1:2])
                        nc.vector.tensor_copy(yvp[:, :, 13:14], yvp[:, :, 12:13])

                        # bilinear x8 horizontal -> yh [C, 9216] f16.
                        # Row-split: DVE rows 0-47, Pool rows 48-95 so early
                        # sigmoid chunks unblock as soon as DVE's half lands.
                        dh = pa.tile([C, 96, 13], f16)
                        nc.vector.tensor_sub(dh[:], yvp[:, :, 1:14], yvp[:, :, 0:13])
                        for hf in (0, 1):
                            h0 = 48 * hf
                            for r in range(8):
                                t = (r + 0.5) / 8 - 0.5
                                kr, b = (0, 1 + t) if r < 4 else (1, t)
                                if r % 4 == 3:
                                    dhs_t = pa.tile([C, 48, 12], f16,
                                                    tag="dhs", name="dhs",
                                                    bufs=2)
                                    nc.vector.tensor_scalar(
                                        dhs_t[:], dh[:, h0:h0 + 48, kr:kr + 12],
                                        float(b), None, op0=ALU.mult)
                                    nc.gpsimd.tensor_add(
                                        yh3[:, h0:h0 + 48, r:96:8], dhs_t[:],
                                        yvp[:, h0:h0 + 48, kr:kr + 12])
                                else:
                                    nc.vector.scalar_tensor_tensor(
                                        yh3[:, h0:h0 + 48, r:96:8],
                                        dh[:, h0:h0 + 48, kr:kr + 12], float(b),
                                        yvp[:, h0:h0 + 48, kr:kr + 12],
                                        op0=ALU.mult, op1=ALU.add)

                    # --- A2: gate, x_gated, projections ---
                    with tc.tile_pool(name="pa2", bufs=1) as pa, \
                         tc.tile_pool(name="paps2", bufs=1, space="PSUM") as paps:

                        nc.sync.dma_start(th16[CI:CI + 1, :], delta_io[:])
                        nc.sync.dma_start(phi16[CI:CI + 1, :], ones_io[:])

                        # early own-chunk path: sigmoid -> x_gated (residual +
                        # theta source); unblocks pass 1 early
                        yhc = pa.tile([C, CH], f16)
                        with tc.tile_critical():
                            pid = nc.vector.partition_id()
                            col0 = pid * CH
                            nc.vector.tensor_copy(yhc[:], yh[:, bass.ds(col0, CH)])
                        gtc = pa.tile([C, CH], f16)
                        nc.scalar.activation(gtc[:], yhc[:], AF.Sigmoid)
                        nc.gpsimd.tensor_mul(xres[:], gtc[:], xchsb[:])

                        # theta chunk [CI, CH] -> fp8 (with zero companion)
                        for o0, w in SUBS:
                            tps = paps.tile([CI, 512], f32, tag="prj", name="tps",
                                            bufs=2)
                            nc.tensor.matmul(tps[:, 0:w], twTsb[:],
                                             xres[:, o0:o0 + w],
                                             start=True, stop=True)
                            nc.vector.tensor_copy(th16[0:CI, o0:o0 + w],
                                                  tps[:, 0:w])

                        # E^T = gw^T WwT [C, C]
                        eps = paps.tile([C, 512], f32, tag="prj", name="eps",
                                        bufs=2)
                        nc.tensor.matmul(eps[:, 0:C], gwsb[:], WwTsb[:],
                                         start=True, stop=True)
                        ET16 = pa.tile([C, C], f16)
                        nc.vector.tensor_copy(ET16[:], eps[:, 0:C])

                        def gt_group(gg):
                            gps = paps.tile([128, 8 * C], f32, tag="prj",
                                            name="gps", bufs=2)
                            for u in range(8):
                                j = gg * 8 + u
                                nc.tensor.matmul(gps[:, u * C:(u + 1) * C],
                                                 xg16[:, j * 128:(j + 1) * 128],
                                                 ET16[:], start=True, stop=True)
                            nc.scalar.activation(
                                G16[:, gg * 8 * C:(gg + 1) * 8 * C], gps[:],
                                AF.Copy)

                        # pipeline over 18 512-col chunks:
                        #   sigmoid -> x_gated -> phi -> fp8; interleave
                        #   G-groups and the first NA pass-1 tiles
                        for i in range(18):
                            sl = slice(i * 512, (i + 1) * 512)
                            xc = hp.tile([C, 512], f16, tag="xc", name="xc",
                                         bufs=4)
                            nc.sync.dma_start(xc[:], x16_io[:, sl])
                            gt = pa.tile([C, 512], f16, tag="gt", name="gt",
                                         bufs=3)
                            nc.scalar.activation(gt[:], yh[:, sl], AF.Sigmoid)
                            nc.vector.tensor_mul(xg16[:, sl], gt[:], xc[:])

                            pps = paps.tile([CI, 512], f32, tag="prj",
                                            name="pps", bufs=2)
                            nc.tensor.matmul(pps[:], pwTsb[:], xg16[:, sl],
                                             start=True, stop=True)
                            if i % 3 == 0:
                                nc.scalar.activation(phi16[0:CI, sl], pps[:],
                                                     AF.Copy)
                            else:
                                nc.vector.tensor_copy(phi16[0:CI, sl], pps[:])

                            if i % 2 == 1:
                                gt_group((i - 1) // 2)
                        for j in range(NA):
                            pass1_tile(j)

                    # ---- PASS 1 main + early pass-2 cols 0-1023 ----
                    pm.release()
                    p2e = tc.alloc_tile_pool(name="p2eps", bufs=1,
                                             space="PSUM")
                    yc01 = [None, None]

                    def pass2_pair01(u, hh, first, last):
                        jl = 2 * u + hh * HALF
                        for ci in (0, 1):
                            nc.tensor.matmul(
                                yc01[ci][:], G3[:, jl:jl + 2, :],
                                s_pair(jl, 512 * ci, 512),
                                start=first, stop=last,
                                perf_mode=PM.DoubleRow, skip_group_check=True)

                    for j in range(NA, MT):
                        pass1_tile(j)
                        if j == NA:
                            allreduce(zinA, zoutA, slice(0, HALF))
                            scale_G(0, zoutA, nc.gpsimd)
                            yc01[0] = p2e.tile([64, 512], f32, name="yc0")
                            yc01[1] = p2e.tile([64, 512], f32, name="yc1")
                        if j >= NA + 2 and j % 2 == 0:
                            u = (j - NA - 2) // 2
                            pass2_pair01(u, 0, first=(u == 0), last=False)
                    pass2_pair01(17, 0, first=False, last=False)
                    allreduce(zinB, zoutB, slice(HALF, MT))
                    scale_G(1, zoutB, nc.vector)
                    for ci in (0, 1):
                        o0 = 512 * ci
                        for u in range(HALF // 2):
                            jl = 2 * u + HALF
                            nc.tensor.matmul(
                                yc01[ci][:], G3[:, jl:jl + 2, :],
                                s_pair(jl, o0, 512),
                                start=False, stop=(u == HALF // 2 - 1),
                                perf_mode=PM.DoubleRow, skip_group_check=True)
                        nc.vector.scalar_tensor_tensor(
                            outsb[:, o0:o0 + 512], yc01[ci][:], 1.0 / GK,
                            xres[:, o0:o0 + 512], op0=ALU.mult, op1=ALU.add)
                        nc.sync.dma_start(out_io[:, o0:o0 + 512],
                                          outsb[:, o0:o0 + 512])
                    p2e.release()
                    p1ps.release()

                    # cols 1024-1151 tail
                    p2l = tc.alloc_tile_pool(name="p2lps", bufs=1,
                                             space="PSUM")
                    yc2 = p2l.tile([64, 128], f32, name="yc2")
                    for hh in (0, 1):
                        for u in range(HALF // 2):
                            jl = 2 * u + hh * HALF
                            nc.tensor.matmul(
                                yc2[:], G3[:, jl:jl + 2, :],
                                s_pair(jl, 1024, 128),
                                start=(hh == 0 and u == 0),
                                stop=(hh == 1 and u == HALF // 2 - 1),
                                perf_mode=PM.DoubleRow, skip_group_check=True)
                    nc.vector.scalar_tensor_tensor(
                        outsb[:, 1024:1152], yc2[:], 1.0 / GK,
                        xres[:, 1024:1152], op0=ALU.mult, op1=ALU.add)
                    nc.sync.dma_start(out_io[:, 1024:1152],
                                      outsb[:, 1024:1152])
                    p2l.release()

    nc.compile()
    return nc


def get_program():
    if "nc" not in _compiled:
        _compiled["nc"] = _build()
    return _compiled["nc"]


def make_in_maps(inputs):
    f16 = np.float16
    x = np.asarray(inputs["x"], np.float32).reshape(C, H, W)
    xflat = np.ascontiguousarray(x.reshape(C, N))
    xpad = np.zeros((C, 98, 98), f16)
    xpad[:, 1:97, 1:97] = x.astype(f16)

    def conv_w(w):
        # [o, i, dy, dx] -> [i, (dy dx), o]
        return np.ascontiguousarray(
            np.asarray(w, np.float32).transpose(1, 2, 3, 0).reshape(C, 9 * C)
        ).astype(f16)

    base = {
        "xpad": xpad,
        "ones_row": np.ones((1, N), f16),
        "delta_row": np.full((1, CH), DELTA, f16),
        "x16": xflat.astype(f16),
        "w1": conv_w(inputs["d1_w"]),
        "w2": conv_w(inputs["d2_w"]),
        "w3": conv_w(inputs["d3_w"]),
        "twT": np.ascontiguousarray(
            np.asarray(inputs["th_w"], np.float32)[:, :, 0, 0].T).astype(f16),
        "pwT": np.ascontiguousarray(
            np.asarray(inputs["ph_w"], np.float32)[:, :, 0, 0].T).astype(f16),
        "gw": np.ascontiguousarray(
            np.asarray(inputs["g_w"], np.float32)[:, :, 0, 0]).astype(f16),
        "WwT": np.ascontiguousarray(
            np.asarray(inputs["W_w"], np.float32)[:, :, 0, 0].T).astype(f16),
    }
    in_maps = []
    for k in range(NCORES):
        m = dict(base)
        m["xch"] = np.ascontiguousarray(
            xflat[:, k * CH:(k + 1) * CH]).astype(f16)
        in_maps.append(m)
    return in_maps


def kernel(**inputs):
    from concourse import bass_utils

    nc = get_program()
    in_maps = make_in_maps(inputs)
    res = bass_utils.run_bass_kernel_spmd(nc, in_maps,
                                          core_ids=list(range(NCORES)))
    out = np.concatenate([res.results[k]["out"] for k in range(NCORES)], axis=1)
    return out.reshape(1, C, H, W).astype(np.float32)
